# revision 1
# baseline (speedup 1.0000x reference)
"""BiLSTM on 8 TRN2 cores — step B: 8-way gate-split recurrence with per-step
cross-core h all-gather via remote_dma_broadcast.  Raw bass (no Tile).

Sharding: every core runs BOTH directions.  Core r owns H-dims
[128r, 128r+128) of both directions: it computes that slice of all four
gates (host reorders gate rows to [i|f|o|g~] so sigmoid is one contiguous
span), updates c/h for its 128 dims, and broadcasts its h^T chunk [128, 64]
bf16 to all 8 cores each step.  The two directions ping-pong so the
broadcast of one direction hides under the compute of the other.

Phase 1 (per direction): xg = x @ W_ih_slice^T + bias_slice, a plain GEMM
(x^T tiles via DMA-transpose of host-cast bf16 x), xg stored time-major in
DRAM scratch.  Phase 2: the recurrence.
"""

import sys
import time

import numpy as np
import ml_dtypes

sys.path.insert(0, "/opt/trn_rl_repo")

import concourse.bass as bass
import concourse.mybir as mybir
from concourse import bacc
from concourse.bass import ds, ts
from concourse.bass_utils import run_bass_kernel_spmd

F32 = mybir.dt.float32
BF16 = mybir.dt.bfloat16
AF = mybir.ActivationFunctionType
OP = mybir.AluOpType
BF16_NP = ml_dtypes.bfloat16

B, S_FULL, I_IN, H = 64, 512, 1024, 1024
NSL = 512            # gate slice per core (128 of each gate)
HSL = 128            # h dims per core
NCORES = 8


def build(S=S_FULL):
    KI = I_IN // 128   # 8
    KH = H // 128      # 8
    TCH = S // 128     # s-quarters per b row in phase 1
    NCH = B * TCH      # chunks per direction in phase 1

    nc = bacc.Bacc("TRN2", target_bir_lowering=False, debug=False,
                   num_devices=NCORES)

    # ---- DRAM ----
    x_d = {}
    wihT_d = {}
    whhT_d = {}
    bias_d = {}
    hout_d = {}
    xg_d = {}
    for d in "fb":
        x_d[d] = nc.dram_tensor(f"x{d}", [B, S, I_IN], BF16, kind="ExternalInput")
        wihT_d[d] = nc.dram_tensor(f"wihT{d}", [I_IN, NSL], BF16, kind="ExternalInput")
        whhT_d[d] = nc.dram_tensor(f"whhT{d}", [H, NSL], BF16, kind="ExternalInput")
        bias_d[d] = nc.dram_tensor(f"bias{d}", [1, NSL], BF16, kind="ExternalInput")
        hout_d[d] = nc.dram_tensor(f"h{d}", [B, S, HSL], F32, kind="ExternalOutput")
        xg_d[d] = nc.dram_tensor(f"xg{d}", [S * B, NSL], BF16, kind="Internal")

    # ---- semaphores ----
    sem = {}
    def SEM(name):
        sem[name] = nc.alloc_semaphore(name)
        return sem[name]
    for d in "fb":
        for nm in ("mm", "add", "act", "c", "tc", "h", "T", "cast", "prep"):
            SEM(f"{nm}_{d}")
        for p in range(2):
            SEM(f"r_{d}{p}"); SEM(f"l_{d}{p}"); SEM(f"shd_{d}{p}")
        for m in range(3):
            SEM(f"sxg_{d}{m}")
    for nm in ("sxT0", "sxT1", "sxT2", "sxT3", "mm1", "evac1", "p1out", "sw",
               "initv", "initg"):
        SEM(nm)

    # ---- SBUF persistent ----
    sb = nc.alloc_sbuf_tensor
    whhT_sb = {d: sb(f"whhT_sb{d}", [128, KH * NSL], BF16).ap() for d in "fb"}
    wihT_sb = {d: sb(f"wihT_sb{d}", [128, KI * NSL], BF16).ap() for d in "fb"}
    bias_sb = {d: sb(f"bias_sb{d}", [1, NSL], BF16).ap() for d in "fb"}
    ones_sb = sb("ones_sb", [1, 128], BF16).ap()
    ident = sb("ident", [64, 64], F32).ap()
    rcv = {d: [sb(f"rcv{d}{p}", [128, KH * B], BF16).ap() for p in range(2)]
           for d in "fb"}
    snd = {d: [sb(f"snd{d}{p}", [128, B], BF16).ap() for p in range(2)]
           for d in "fb"}
    xgb = {d: [sb(f"xgb{d}{m}", [B, NSL], BF16).ap() for m in range(3)]
           for d in "fb"}
    gadd = {d: sb(f"gadd{d}", [B, NSL], F32).ap() for d in "fb"}
    acts = {d: sb(f"acts{d}", [B, NSL], F32).ap() for d in "fb"}
    c_sb = {d: sb(f"c{d}", [B, HSL], F32).ap() for d in "fb"}
    tnc = {d: sb(f"tnc{d}", [B, HSL], F32).ap() for d in "fb"}
    t1_sb = {d: sb(f"t1{d}", [B, HSL], F32).ap() for d in "fb"}
    t2_sb = {d: sb(f"t2{d}", [B, HSL], F32).ap() for d in "fb"}
    hbuf = {d: [sb(f"hb{d}{p}", [B, HSL], F32).ap() for p in range(2)]
            for d in "fb"}
    xT = [sb(f"xT{m}", [128, KI * 128], BF16).ap() for m in range(4)]
    ot = [sb(f"ot{m}", [128, NSL], BF16).ap() for m in range(2)]

    # ---- PSUM static ----
    ap_ = nc.alloc_psum_tensor
    ps1 = [ap_(f"ps1{m}", [128, NSL], F32).ap() for m in range(2)]
    g_ps = {d: ap_(f"gps{d}", [B, NSL], F32).ap() for d in "fb"}
    tps = {d: [ap_(f"tps{d}{p}", [128, B], F32).ap() for p in range(2)]
           for d in "fb"}

    # ---- prologue ----
    for d in "fb":
        nc.sync.dma_start(
            whhT_sb[d].rearrange("p (k n) -> p k n", n=NSL),
            whhT_d[d].ap().rearrange("(k p) n -> p k n", p=128),
        ).then_inc(sem["sw"], 16)
        nc.sync.dma_start(
            wihT_sb[d].rearrange("p (k n) -> p k n", n=NSL),
            wihT_d[d].ap().rearrange("(k p) n -> p k n", p=128),
        ).then_inc(sem["sw"], 16)
        nc.sync.dma_start(bias_sb[d], bias_d[d].ap()).then_inc(sem["sw"], 16)

    nc.vector.memset(ones_sb, 1.0).then_inc(sem["initv"], 1)
    for d in "fb":
        nc.vector.memset(rcv[d][0], 0.0).then_inc(sem["initv"], 1)
        nc.vector.memset(c_sb[d], 0.0).then_inc(sem["initv"], 1)
    # identity for PE transpose (f32)
    nc.gpsimd.memset(ident, 0.0)
    nc.gpsimd.affine_select(
        out=ident, in_=ident, compare_op=OP.not_equal, fill=1.0,
        base=0, pattern=[[-1, 64]], channel_multiplier=1,
    ).then_inc(sem["initg"], 1)
    pid = nc.gpsimd.partition_id()

    # PE waits once for all the setup
    nc.tensor.wait_ge(sem["sw"], 16 * 6)
    nc.tensor.wait_ge(sem["initv"], 5)
    nc.tensor.wait_ge(sem["initg"], 1)

    # ---- phase 1: xg[d] = x[d] @ wihT[d] + bias[d]  (time-major out) ----
    cidx = 0
    for d in "fb":
        xg3 = xg_d[d].ap().rearrange("(s b) n -> s b n", b=B)
        for b in range(B):
            for sq in range(TCH):
                m2 = cidx % 2
                m4 = cidx % 4
                sxT = sem[f"sxT{m4}"]
                use = cidx // 4 + 1
                # in-DMAs (transpose): x[b, s-slice, k-chunk] -> xT[m4][:, k]
                if cidx >= 4:
                    nc.sync.wait_ge(sem["mm1"], cidx - 3)
                for k in range(KI):
                    nc.sync.dma_start(
                        xT[m4][:, ts(k, 128)],
                        x_d[d].ap()[b, ds(128 * sq, 128), ts(k, 128)],
                        transpose=True,
                    ).then_inc(sxT, 16)
                # matmuls
                nc.tensor.wait_ge(sxT, 128 * use)
                if cidx >= 2:
                    nc.tensor.wait_ge(sem["evac1"], cidx - 1)
                for k in range(KI):
                    nc.tensor.matmul(ps1[m2], xT[m4][:, ts(k, 128)],
                                     wihT_sb[d][:, ts(k, NSL)],
                                     start=(k == 0), stop=False)
                nc.tensor.matmul(ps1[m2], ones_sb, bias_sb[d],
                                 start=False, stop=True).then_inc(sem["mm1"], 1)
                # evac
                nc.vector.wait_ge(sem["mm1"], cidx + 1)
                nc.vector.tensor_copy(ot[m2], ps1[m2]).then_inc(sem["evac1"], 1)
                # out
                nc.sync.wait_ge(sem["evac1"], cidx + 1)
                nc.sync.dma_start(xg3[ds(128 * sq, 128), b, :],
                                  ot[m2]).then_inc(sem["p1out"], 16)
                cidx += 1

    # ---- phase 2 ----
    RD = [(0, k) for k in range(NCORES)]
    # xg prefetch for steps 0..2 (after all phase-1 writes land)
    nc.sync.wait_ge(sem["p1out"], 16 * cidx)
    for d in "fb":
        for u in range(min(3, S)):
            nc.sync.dma_start(xgb[d][u], xg_d[d].ap()[ds(B * u, B), :]
                              ).then_inc(sem[f"sxg_{d}{u}"], 16)

    ho2 = {d: hout_d[d].ap().rearrange("b s h -> b (s h)") for d in "fb"}

    for t in range(S):
        p = t % 2
        m3 = t % 3
        # ---------- SP: xg prefetch t+3, hout t ----------
        for d in "fb":
            if t + 3 < S:
                nc.sync.wait_ge(sem[f"add_{d}"], t + 1)
                nc.sync.dma_start(xgb[d][m3],
                                  xg_d[d].ap()[ds(B * (t + 3), B), :]
                                  ).then_inc(sem[f"sxg_{d}{m3}"], 16)
        # ---------- PE: matmuls ----------
        for d in "fb":
            if t >= 1:
                nc.tensor.wait_ge(sem[f"r_{d}{p}"], 16 * ((t + 1) // 2))
                nc.tensor.wait_ge(sem[f"add_{d}"], t)
            for k in range(KH):
                mm = nc.tensor.matmul(g_ps[d], rcv[d][p][:, ts(k, B)],
                                      whhT_sb[d][:, ts(k, NSL)],
                                      start=(k == 0), stop=(k == KH - 1))
            mm.then_inc(sem[f"mm_{d}"], 1)
        # ---------- DVE: gate add ----------
        for d in "fb":
            nc.vector.wait_ge(sem[f"mm_{d}"], t + 1)
            nc.vector.wait_ge(sem[f"sxg_{d}{m3}"], 16 * (t // 3 + 1))
            nc.vector.tensor_tensor(gadd[d], g_ps[d], xgb[d][m3],
                                    op=OP.add).then_inc(sem[f"add_{d}"], 1)
        # ---------- ACT: activations ----------
        for d in "fb":
            nc.scalar.wait_ge(sem[f"add_{d}"], t + 1)
            nc.scalar.activation(acts[d][:, ds(0, 384)], gadd[d][:, ds(0, 384)],
                                 AF.Sigmoid)
            nc.scalar.activation(acts[d][:, ds(384, 128)],
                                 gadd[d][:, ds(384, 128)],
                                 AF.Tanh).then_inc(sem[f"act_{d}"], 1)
        # ---------- DVE: c update ----------
        for d in "fb":
            nc.vector.wait_ge(sem[f"act_{d}"], t + 1)
            nc.vector.tensor_tensor(t1_sb[d], acts[d][:, ds(128, 128)],
                                    c_sb[d], op=OP.mult)
            nc.vector.tensor_tensor(t2_sb[d], acts[d][:, ds(0, 128)],
                                    acts[d][:, ds(384, 128)], op=OP.mult)
            nc.vector.tensor_tensor(c_sb[d], t1_sb[d], t2_sb[d],
                                    op=OP.add).then_inc(sem[f"c_{d}"], 1)
        # ---------- ACT: tanh(c) ----------
        for d in "fb":
            nc.scalar.wait_ge(sem[f"c_{d}"], t + 1)
            nc.scalar.activation(tnc[d], c_sb[d],
                                 AF.Tanh).then_inc(sem[f"tc_{d}"], 1)
        # ---------- DVE: h ----------
        for d in "fb":
            nc.vector.wait_ge(sem[f"tc_{d}"], t + 1)
            if t >= 2:
                nc.vector.wait_ge(sem[f"shd_{d}{p}"], 16 * (t // 2))
            nc.vector.tensor_tensor(hbuf[d][p], acts[d][:, ds(256, 128)],
                                    tnc[d], op=OP.mult
                                    ).then_inc(sem[f"h_{d}"], 1)
        # ---------- SP: hout ----------
        for d in "fb":
            nc.sync.wait_ge(sem[f"h_{d}"], t + 1)
            nc.sync.dma_start(ho2[d][:, ds(t * HSL, HSL)], hbuf[d][p]
                              ).then_inc(sem[f"shd_{d}{p}"], 16)
        # ---------- PE: transpose h ----------
        for d in "fb":
            nc.tensor.wait_ge(sem[f"h_{d}"], t + 1)
            if t >= 2:
                nc.tensor.wait_ge(sem[f"cast_{d}"], t - 1)
            nc.tensor.transpose(tps[d][p], hbuf[d][p],
                                ident).then_inc(sem[f"T_{d}"], 1)
        # ---------- ACT: cast h^T -> bf16 snd (keeps DVE off the path) ----------
        for d in "fb":
            nc.scalar.wait_ge(sem[f"T_{d}"], t + 1)
            if t >= 2:
                nc.scalar.wait_ge(sem[f"l_{d}{p}"], 16 * (t // 2))
            nc.scalar.activation(snd[d][p], tps[d][p],
                                 AF.Copy).then_inc(sem[f"cast_{d}"], 1)
        # ---------- POOL: broadcast ----------
        for d in "fb":
            nc.gpsimd.remote_dma_broadcast(
                rcv[d][(t + 1) % 2][:, ds(pid * B, B)], snd[d][p],
                remote_sem=sem[f"r_{d}{(t + 1) % 2}"],
                local_sem=sem[f"l_{d}{p}"],
                rdests=RD).then_inc(sem[f"prep_{d}"], 1)
        for d in "fb":
            nc.gpsimd.wait_ge(sem[f"prep_{d}"], t + 1)
            nc.gpsimd.wait_ge(sem[f"cast_{d}"], t + 1)
            nc.gpsimd.trigger_dma(count=1)

    # ---- epilogue: drain all async traffic before NEFF end ----
    assert S % 2 == 0
    for d in "fb":
        for p in range(2):
            nc.sync.wait_ge(sem[f"shd_{d}{p}"], 16 * (S // 2))
            nc.sync.wait_ge(sem[f"l_{d}{p}"], 16 * (S // 2))
            nc.sync.wait_ge(sem[f"r_{d}{p}"], 16 * (S // 2))

    nc.compile()
    nc.has_collectives = True  # force PJRT co-scheduling
    return nc


_CACHE = {}


def _get(S):
    if S not in _CACHE:
        _CACHE[S] = build(S)
    return _CACHE[S]


def _host_shard(inputs, S):
    fx = np.asarray(inputs["forward_x"], np.float32)[:, :S]
    bx = np.asarray(inputs["backward_x"], np.float32)[:, :S]
    xf = np.ascontiguousarray(fx).astype(BF16_NP)
    xb = np.ascontiguousarray(bx[:, ::-1]).astype(BF16_NP)
    maps = []
    for r in range(NCORES):
        rows = np.concatenate([
            np.arange(128 * r, 128 * r + 128),             # i
            np.arange(H + 128 * r, H + 128 * r + 128),     # f
            np.arange(3 * H + 128 * r, 3 * H + 128 * r + 128),  # o
            np.arange(2 * H + 128 * r, 2 * H + 128 * r + 128),  # g~
        ])
        m = {"xf": xf, "xb": xb}
        for d, sfx in (("f", "_f"), ("b", "_b")):
            wih = np.asarray(inputs[f"W_ih{sfx}"], np.float32)[rows]
            whh = np.asarray(inputs[f"W_hh{sfx}"], np.float32)[rows]
            bias = (np.asarray(inputs[f"b_ih{sfx}"], np.float32)
                    + np.asarray(inputs[f"b_hh{sfx}"], np.float32))[rows]
            m[f"wihT{d}"] = np.ascontiguousarray(wih.T).astype(BF16_NP)
            m[f"whhT{d}"] = np.ascontiguousarray(whh.T).astype(BF16_NP)
            m[f"bias{d}"] = bias.reshape(1, -1).astype(BF16_NP)
        maps.append(m)
    return maps


def run(inputs, S=S_FULL, trace=False, **_):
    maps = _host_shard(inputs, S)
    nc = _get(S)
    t0 = time.time()
    res = run_bass_kernel_spmd(nc, maps, core_ids=list(range(NCORES)),
                               trace=trace)
    wall = time.time() - t0
    outs = res.results
    fwd = np.concatenate([outs[r]["hf"] for r in range(NCORES)], axis=2)
    bwd = np.concatenate([outs[r]["hb"] for r in range(NCORES)], axis=2)[:, ::-1]
    return (fwd, bwd), res, wall


def kernel(**inputs):
    (fwd, bwd), _, _ = run(inputs)
    return fwd.astype(np.float32), bwd.astype(np.float32)


def run_timed(inputs, S=S_FULL, iters=3):
    """Mirror bass2jax.run_bass_via_pjrt but pre-stage device inputs and time
    pure execution (incl. PJRT dispatch, excl. H2D of the big tensors)."""
    import jax
    import jax.numpy as jnp
    from jax.sharding import Mesh, PartitionSpec
    from jax.experimental.shard_map import shard_map
    import concourse.mybir as mb
    from concourse.bass2jax import (_bass_exec_p, partition_id_tensor,
                                    install_neuronx_cc_hook)

    maps = _host_shard(inputs, S)
    nc = _get(S)
    install_neuronx_cc_hook()

    partition_name = nc.partition_id_tensor.name if nc.partition_id_tensor else None
    in_names, out_names, out_avals, zero_outs = [], [], [], []
    for alloc in nc.m.functions[0].allocations:
        if not isinstance(alloc, mb.MemoryLocationSet):
            continue
        name = alloc.memorylocations[0].name
        if alloc.kind == "ExternalInput":
            if name != partition_name:
                in_names.append(name)
        elif alloc.kind == "ExternalOutput":
            out_names.append(name)
            shape = tuple(alloc.tensor_shape)
            dtype = mb.dt.np(alloc.dtype)
            out_avals.append(jax.core.ShapedArray(shape, dtype))
            zero_outs.append(np.zeros(shape, dtype))
    n_params = len(in_names)
    n_outs = len(out_avals)
    all_in_names = list(in_names) + out_names
    if partition_name is not None:
        all_in_names.append(partition_name)
    donate = tuple(range(n_params, n_params + n_outs))

    def _body(*args):
        operands = list(args)
        if partition_name is not None:
            operands.append(partition_id_tensor())
        return tuple(_bass_exec_p.bind(
            *operands, out_avals=tuple(out_avals), in_names=tuple(all_in_names),
            out_names=tuple(out_names), lowering_input_output_aliases=(),
            sim_require_finite=True, sim_require_nnan=True, nc=nc))

    devices = jax.devices()[:NCORES]
    mesh = Mesh(np.asarray(devices), ("core",))
    in_specs = (PartitionSpec("core"),) * (n_params + n_outs)
    out_specs = (PartitionSpec("core"),) * n_outs
    sharded = jax.jit(shard_map(_body, mesh=mesh, in_specs=in_specs,
                                out_specs=out_specs, check_rep=False),
                      donate_argnums=donate, keep_unused=True)
    sharding = jax.sharding.NamedSharding(mesh, PartitionSpec("core"))
    concat_in = [
        jax.device_put(
            np.concatenate([np.asarray(maps[c][nm]) for c in range(NCORES)],
                           axis=0), sharding)
        for nm in in_names]
    jax.block_until_ready(concat_in)

    times = []
    out_arrs = None
    for it in range(iters):
        zeros = [jax.device_put(
            np.zeros((NCORES * z.shape[0], *z.shape[1:]), z.dtype), sharding)
            for z in zero_outs]
        jax.block_until_ready(zeros)
        t0 = time.time()
        out_arrs = sharded(*concat_in, *zeros)
        jax.block_until_ready(out_arrs)
        times.append(time.time() - t0)
    res = {name: np.asarray(out_arrs[i]).reshape(NCORES, *out_avals[i].shape)
           for i, name in enumerate(out_names)}
    fwd = np.concatenate([res["hf"][r] for r in range(NCORES)], axis=2)
    bwd = np.concatenate([res["hb"][r] for r in range(NCORES)], axis=2)[:, ::-1]
    return (fwd, bwd), times



# revision 2
# speedup vs baseline: 4288.6890x; 4288.6890x over previous
"""BiLSTM on 8 TRN2 cores — v2: single fused loop.

Gate-split 8 ways (each core owns 128 h-dims of both directions; gates
reordered [i|f|o|g~] on host so sigmoid spans one contiguous block).
Differences vs v1:
  - x is pre-transposed on HOST to [KI, 128, S, B] bf16, so the input
    projection (xg) needs no DMA transposes: plain contiguous loads.
  - xg is computed ON THE FLY by the PE 6 steps ahead of the recurrence
    (no DRAM xg scratch, no separate phase 1) -> PE stays warm (HAM),
    broadcast-latency bubbles are filled with xg GEMMs, and cross-core
    phase-1 skew disappears.
  - xg is accumulated into the gate PSUM by an identity matmul on the PE
    (kills the DVE gate-add) and activations read PSUM directly.
  - h is produced in bf16; hout DRAM tensor is bf16 (host upcasts).
"""

import sys
import time

import numpy as np
import ml_dtypes

sys.path.insert(0, "/opt/trn_rl_repo")

import concourse.bass as bass
import concourse.mybir as mybir
from concourse import bacc
from concourse.bass import ds, ts
from concourse.bass_utils import run_bass_kernel_spmd

F32 = mybir.dt.float32
BF16 = mybir.dt.bfloat16
AF = mybir.ActivationFunctionType
OP = mybir.AluOpType
BF16_NP = ml_dtypes.bfloat16

B, S_FULL, I_IN, H = 64, 512, 1024, 1024
NSL = 512            # gate slice per core (128 of each gate)
HSL = 128            # h dims per core
KI = I_IN // 128     # 8
KH = H // 128        # 8
NCORES = 8
WSTEPS = 8           # steps per x-tile window
LOOK = 3             # xg pairs of lookahead (pair = 2 steps)


def build(S=S_FULL):
    assert S % WSTEPS == 0 and S % 2 == 0
    NW = S // WSTEPS         # x windows
    NP = S // 2              # xg pairs

    nc = bacc.Bacc("TRN2", target_bir_lowering=False, debug=False,
                   num_devices=NCORES)

    # ---- DRAM ----
    xk_d, wihT_d, whhT_d, bias_d, hout_d = {}, {}, {}, {}, {}
    for d in "fb":
        xk_d[d] = nc.dram_tensor(f"xk{d}", [KI, 128, S, B], BF16,
                                 kind="ExternalInput")
        wihT_d[d] = nc.dram_tensor(f"wihT{d}", [I_IN, NSL], BF16,
                                   kind="ExternalInput")
        whhT_d[d] = nc.dram_tensor(f"whhT{d}", [H, NSL], BF16,
                                   kind="ExternalInput")
        bias_d[d] = nc.dram_tensor(f"bias{d}", [1, NSL], BF16,
                                   kind="ExternalInput")
        hout_d[d] = nc.dram_tensor(f"h{d}", [B, S, HSL], F32,
                                   kind="ExternalOutput")

    # ---- semaphores ----
    sem = {}
    def SEM(name):
        sem[name] = nc.alloc_semaphore(name)
        return sem[name]
    for d in "fb":
        for nm in ("sxt", "xgm", "xge", "mm", "act", "act2", "cdon", "tc",
                   "hdon", "T", "cast", "prep"):
            SEM(f"{nm}_{d}")
        for p in range(2):
            SEM(f"r_{d}{p}"); SEM(f"l_{d}{p}"); SEM(f"shd_{d}{p}")
    for nm in ("sw", "initv", "initg"):
        SEM(nm)

    # ---- SBUF ----
    sb = nc.alloc_sbuf_tensor
    whhT_sb = {d: sb(f"whhT_sb{d}", [128, KH * NSL], BF16).ap() for d in "fb"}
    wihT_sb = {d: sb(f"wihT_sb{d}", [128, KI * NSL], BF16).ap() for d in "fb"}
    bias_sb = {d: sb(f"bias_sb{d}", [1, NSL], BF16).ap() for d in "fb"}
    ones_sb = sb("ones_sb", [1, 128], BF16).ap()
    ident = sb("ident", [64, 64], BF16).ap()        # for xg psum-accumulate
    identf = sb("identf", [64, 64], F32).ap()       # for h transpose
    xt = {d: [sb(f"xt{d}{q}", [128, KI * WSTEPS * B], BF16).ap()
              for q in range(3)] for d in "fb"}
    xgr = {d: [[sb(f"xgr{d}{j}{hf}", [64, NSL], BF16).ap() for hf in range(2)]
               for j in range(4)] for d in "fb"}
    rcv = {d: [sb(f"rcv{d}{p}", [128, KH * B], BF16).ap() for p in range(2)]
           for d in "fb"}
    snd = {d: [sb(f"snd{d}{p}", [128, B], BF16).ap() for p in range(2)]
           for d in "fb"}
    acts = {d: sb(f"acts{d}", [64, NSL], F32).ap() for d in "fb"}
    c_sb = {d: sb(f"c{d}", [64, HSL], F32).ap() for d in "fb"}
    t1_sb = {d: sb(f"t1{d}", [64, HSL], F32).ap() for d in "fb"}
    t2_sb = {d: sb(f"t2{d}", [64, HSL], F32).ap() for d in "fb"}
    tnc = {d: sb(f"tnc{d}", [64, HSL], F32).ap() for d in "fb"}
    h_sb = {d: [sb(f"h{d}{p}", [64, HSL], F32).ap() for p in range(2)]
            for d in "fb"}

    # ---- PSUM ----
    ap_ = nc.alloc_psum_tensor
    g_ps = {d: ap_(f"gps{d}", [64, NSL], F32).ap() for d in "fb"}
    xps = {d: [ap_(f"xps{d}{q}", [128, NSL], F32).ap() for q in range(2)]
           for d in "fb"}
    tps_all = ap_("tps", [128, 128], F32).ap()
    tps = {"f": tps_all[:, ds(0, 64)], "b": tps_all[:, ds(64, 64)]}

    RD = [(0, k) for k in range(NCORES)]

    # ================= prologue =================
    for d in "fb":
        nc.sync.dma_start(
            whhT_sb[d].rearrange("p (k n) -> p k n", n=NSL),
            whhT_d[d].ap().rearrange("(k p) n -> p k n", p=128),
        ).then_inc(sem["sw"], 16)
        nc.sync.dma_start(
            wihT_sb[d].rearrange("p (k n) -> p k n", n=NSL),
            wihT_d[d].ap().rearrange("(k p) n -> p k n", p=128),
        ).then_inc(sem["sw"], 16)
        nc.sync.dma_start(bias_sb[d], bias_d[d].ap()).then_inc(sem["sw"], 16)

    def win_dma(d, w, ks):
        # window w: steps [8w, 8w+8), chunks ks
        for k in ks:
            nc.sync.dma_start(
                xt[d][w % 3][:, ds(k * WSTEPS * B, WSTEPS * B)]
                .rearrange("p (s b) -> p s b", b=B),
                xk_d[d].ap()[k, :, ds(WSTEPS * w, WSTEPS), :],
            ).then_inc(sem[f"sxt_{d}"], 16)

    for w in range(min(3, NW)):
        for d in "fb":
            win_dma(d, w, range(KI))

    nc.vector.memset(ones_sb, 1.0).then_inc(sem["initv"], 1)
    for id_ap in (ident, identf):
        nc.gpsimd.memset(id_ap, 0.0)
        nc.gpsimd.affine_select(
            out=id_ap, in_=id_ap, compare_op=OP.not_equal, fill=1.0,
            base=0, pattern=[[-1, 64]], channel_multiplier=1,
        ).then_inc(sem["initg"], 1)
    pid = nc.gpsimd.partition_id()

    nc.tensor.wait_ge(sem["sw"], 16 * 6)
    nc.tensor.wait_ge(sem["initv"], 1)
    nc.tensor.wait_ge(sem["initg"], 2)

    def xg_gen(d, j):
        # PE: xg for steps (2j, 2j+1) into xps[d][j%2]
        w = (2 * j) // WSTEPS
        sl = 2 * j - WSTEPS * w          # s offset inside window
        nc.tensor.wait_ge(sem[f"sxt_{d}"], 16 * KI * (w + 1))
        if j >= 2:
            nc.tensor.wait_ge(sem[f"xge_{d}"], j - 1)
        for k in range(KI):
            nc.tensor.matmul(xps[d][j % 2],
                             xt[d][w % 3][:, ds(k * WSTEPS * B + sl * B, 128)],
                             wihT_sb[d][:, ts(k, NSL)],
                             start=(k == 0), stop=False)
        nc.tensor.matmul(xps[d][j % 2], ones_sb, bias_sb[d],
                         start=False, stop=True).then_inc(sem[f"xgm_{d}"], 1)

    def xg_evac(d, j):
        # DVE: psum -> bf16 sbuf ring (two 64-partition halves)
        nc.vector.wait_ge(sem[f"xgm_{d}"], j + 1)
        nc.vector.tensor_copy(xgr[d][j % 4][0], xps[d][j % 2][ds(0, 64), :])
        nc.vector.tensor_copy(xgr[d][j % 4][1], xps[d][j % 2][ds(64, 64), :]
                              ).then_inc(sem[f"xge_{d}"], 1)

    # prologue xg pairs 0..LOOK-1
    for j in range(min(LOOK, NP)):
        for d in "fb":
            xg_gen(d, j)
    for j in range(min(LOOK, NP)):
        for d in "fb":
            xg_evac(d, j)

    # ================= main loop =================
    for t in range(S):
        p = t % 2
        # ---------- SYNC ----------
        for d in "fb":
            nc.sync.wait_ge(sem[f"hdon_{d}"], t + 1)
            nc.sync.dma_start(hout_d[d].ap()[:, t, :], h_sb[d][p]
                              ).then_inc(sem[f"shd_{d}{p}"], 16)
        # x-window loads spread one chunk per dir per step (avoids a 16-DMA
        # burst on the Sync queue every 8th step, which stalled gen/hout)
        if t >= 2:
            u = t - 2
            w = u // WSTEPS + 3
            k = u % WSTEPS
            if w < NW:
                for d in "fb":
                    nc.sync.wait_ge(sem[f"xgm_{d}"], 4 * (w - 3) + 4)
                    win_dma(d, w, [k])

        # ---------- PE ----------
        # xg lookahead: f pairs on even steps, b pairs on odd steps, so the
        # broadcast-latency bubble has PE work EVERY step (keeps HAM warm).
        gd = "f" if t % 2 == 0 else "b"
        gj = t // 2 + LOOK
        if gj < NP:
            xg_gen(gd, gj)
        # xg accumulate first (no cross-core dep): opens the psum group early
        # and keeps it off the broadcast-gated critical path.
        for d in "fb":
            if t >= 1:
                nc.tensor.wait_ge(sem[f"act2_{d}"], t)
            nc.tensor.wait_ge(sem[f"xge_{d}"], t // 2 + 1)
            mmi = nc.tensor.matmul(g_ps[d], ident,
                                   xgr[d][(t // 2) % 4][t % 2],
                                   start=True, stop=(t == 0),
                                   skip_group_check=True)
            if t == 0:
                mmi.then_inc(sem[f"mm_{d}"], 1)
        for d in "fb":
            if t >= 1:
                nc.tensor.wait_ge(sem[f"r_{d}{p}"], 16 * ((t + 1) // 2))
                for k in range(KH):
                    mmi = nc.tensor.matmul(g_ps[d], rcv[d][p][:, ts(k, B)],
                                           whhT_sb[d][:, ts(k, NSL)],
                                           start=False, stop=(k == KH - 1),
                                           skip_group_check=True)
                mmi.then_inc(sem[f"mm_{d}"], 1)
        for d in "fb":
            nc.tensor.wait_ge(sem[f"hdon_{d}"], t + 1)
            if t >= 1:
                nc.tensor.wait_ge(sem[f"cast_{d}"], t)
            nc.tensor.transpose(tps[d], h_sb[d][p],
                                identf).then_inc(sem[f"T_{d}"], 1)

        # ---------- ACT ----------
        # sigmoid split: i,f first (starts the c-chain earlier), o after
        # tanh(g~) — o is only needed for h, gated transitively by tanhc.
        for d in "fb":
            nc.scalar.wait_ge(sem[f"mm_{d}"], t + 1)
            nc.scalar.activation(acts[d][:, ds(0, 256)],
                                 g_ps[d][:, ds(0, 256)], AF.Sigmoid)
            nc.scalar.activation(acts[d][:, ds(384, 128)],
                                 g_ps[d][:, ds(384, 128)],
                                 AF.Tanh).then_inc(sem[f"act_{d}"], 1)
            nc.scalar.activation(acts[d][:, ds(256, 128)],
                                 g_ps[d][:, ds(256, 128)],
                                 AF.Sigmoid).then_inc(sem[f"act2_{d}"], 1)
            nc.scalar.wait_ge(sem[f"cdon_{d}"], t + 1)
            nc.scalar.activation(tnc[d], c_sb[d],
                                 AF.Tanh).then_inc(sem[f"tc_{d}"], 1)

        # ---------- DVE ----------
        def c_chain(d):
            nc.vector.wait_ge(sem[f"act_{d}"], t + 1)
            if t == 0:
                nc.vector.tensor_tensor(c_sb[d], acts[d][:, ds(0, 128)],
                                        acts[d][:, ds(384, 128)],
                                        op=OP.mult).then_inc(sem[f"cdon_{d}"], 1)
            else:
                nc.vector.tensor_tensor(t1_sb[d], acts[d][:, ds(128, 128)],
                                        c_sb[d], op=OP.mult)
                nc.vector.tensor_tensor(t2_sb[d], acts[d][:, ds(0, 128)],
                                        acts[d][:, ds(384, 128)], op=OP.mult)
                nc.vector.tensor_tensor(c_sb[d], t1_sb[d], t2_sb[d],
                                        op=OP.add).then_inc(sem[f"cdon_{d}"], 1)

        def h_mul(d):
            nc.vector.wait_ge(sem[f"tc_{d}"], t + 1)
            if t >= 2:
                nc.vector.wait_ge(sem[f"shd_{d}{p}"], 16 * (t // 2))
            nc.vector.tensor_tensor(h_sb[d][p], acts[d][:, ds(256, 128)],
                                    tnc[d], op=OP.mult
                                    ).then_inc(sem[f"hdon_{d}"], 1)

        def snd_cast(d):
            nc.vector.wait_ge(sem[f"T_{d}"], t + 1)
            if t >= 2:
                nc.vector.wait_ge(sem[f"l_{d}{p}"], 16 * (t // 2))
            nc.vector.tensor_copy(snd[d][p],
                                  tps[d]).then_inc(sem[f"cast_{d}"], 1)

        c_chain("f")
        h_mul("f")
        c_chain("b")
        snd_cast("f")
        h_mul("b")
        snd_cast("b")
        if gj < NP:
            xg_evac(gd, gj)

        # ---------- POOL ----------
        if t < S - 1:
            for d in "fb":
                nc.gpsimd.remote_dma_broadcast(
                    rcv[d][(t + 1) % 2][:, ds(pid * B, B)], snd[d][p],
                    remote_sem=sem[f"r_{d}{(t + 1) % 2}"],
                    local_sem=sem[f"l_{d}{p}"],
                    rdests=RD).then_inc(sem[f"prep_{d}"], 1)
            for d in "fb":
                nc.gpsimd.wait_ge(sem[f"prep_{d}"], t + 1)
                nc.gpsimd.wait_ge(sem[f"cast_{d}"], t + 1)
                nc.gpsimd.trigger_dma(count=1)

    # ================= epilogue =================
    for d in "fb":
        for p in range(2):
            nc.sync.wait_ge(sem[f"shd_{d}{p}"], 16 * (S // 2))
        # bcasts from snd[p]: steps t==p mod 2, t <= S-2
        nl0 = (S - 1 + 1) // 2   # t even <= S-2 : 0..S-2 even count
        nl0 = len(range(0, S - 1, 2))
        nl1 = len(range(1, S - 1, 2))
        nc.sync.wait_ge(sem[f"l_{d}0"], 16 * nl0)
        nc.sync.wait_ge(sem[f"l_{d}1"], 16 * nl1)
        # bcasts into rcv[p]: t with (t+1)%2==p, t <= S-2
        nr1 = len(range(0, S - 1, 2))
        nr0 = len(range(1, S - 1, 2))
        nc.sync.wait_ge(sem[f"r_{d}0"], 16 * nr0)
        nc.sync.wait_ge(sem[f"r_{d}1"], 16 * nr1)

    nc.compile()
    nc.has_collectives = True
    return nc


_CACHE = {}


def _get(S):
    if S not in _CACHE:
        _CACHE[S] = build(S)
    return _CACHE[S]


def _host_shard(inputs, S):
    fx = np.asarray(inputs["forward_x"], np.float32)[:, :S]
    bx = np.asarray(inputs["backward_x"], np.float32)[:, :S]
    xs = {}
    for d, x in (("f", fx), ("b", bx[:, ::-1])):
        xb = np.ascontiguousarray(x).astype(BF16_NP)      # [B, S, I]
        xk = np.ascontiguousarray(
            xb.transpose(2, 1, 0).reshape(KI, 128, S, B))  # [KI,128,S,B]
        xs[d] = xk
    maps = []
    for r in range(NCORES):
        rows = np.concatenate([
            np.arange(128 * r, 128 * r + 128),                   # i
            np.arange(H + 128 * r, H + 128 * r + 128),           # f
            np.arange(3 * H + 128 * r, 3 * H + 128 * r + 128),   # o
            np.arange(2 * H + 128 * r, 2 * H + 128 * r + 128),   # g~
        ])
        m = {"xkf": xs["f"], "xkb": xs["b"]}
        for d, sfx in (("f", "_f"), ("b", "_b")):
            wih = np.asarray(inputs[f"W_ih{sfx}"], np.float32)[rows]
            whh = np.asarray(inputs[f"W_hh{sfx}"], np.float32)[rows]
            bias = (np.asarray(inputs[f"b_ih{sfx}"], np.float32)
                    + np.asarray(inputs[f"b_hh{sfx}"], np.float32))[rows]
            m[f"wihT{d}"] = np.ascontiguousarray(wih.T).astype(BF16_NP)
            m[f"whhT{d}"] = np.ascontiguousarray(whh.T).astype(BF16_NP)
            m[f"bias{d}"] = bias.reshape(1, -1).astype(BF16_NP)
        maps.append(m)
    return maps


def run(inputs, S=S_FULL, trace=False, trace_cores=None, **_):
    maps = _host_shard(inputs, S)
    nc = _get(S)
    t0 = time.time()
    kw = {}
    if trace_cores is not None:
        kw = dict(trace_cores=trace_cores, stitch_traces=True)
    res = run_bass_kernel_spmd(nc, maps, core_ids=list(range(NCORES)),
                               trace=trace, **kw)
    wall = time.time() - t0
    outs = res.results
    fwd = np.concatenate([outs[r]["hf"] for r in range(NCORES)], axis=2)
    bwd = np.concatenate([outs[r]["hb"] for r in range(NCORES)],
                         axis=2)[:, ::-1]
    return (fwd, bwd), res, wall


def kernel(**inputs):
    (fwd, bwd), _, _ = run(inputs)
    return fwd, bwd
